# revision 15
# baseline (speedup 1.0000x reference)
"""Column-parallel linear Y = X @ W^T + b on 8 Trainium2 NeuronCores.

Strategy: sequence-shard X across the 8 cores (4096 tokens each); every core
holds the full weight, computes its token slab against all 4096 output
features, so no collective is needed and no core re-reads another's tokens.

v4 (mixed fp8/bf16): k-tiles 0-1 (256 of 1024 contraction rows) run as ONE
fp8-e4m3 DoubleRow matmul per output tile (the PE packs 2 fp8 weights per
cell, virtualizing the array to 256x128, streaming 2 k-rows/cycle), k-tiles
2-7 run in bf16. Scales (x*0.25, w*4) cancel in the product so both parts
accumulate into the same PSUM group. Measured norm rel err on the reference
case: 1.61e-2 (gate 2e-2). The output returns bf16; the host upcasts.

Device layout (per core):
  xT  [8, 128, 6, 512] bf16  xT[g, p, ko, m'] = X_shard[g*512+m', (ko+2)*128+p]
  wT  [8, 128, 6, 512] bf16  wT[nc, p, ko, q] = W[nc*512+q, (ko+2)*128+p]
  x8T [128, 2, 4096]   fp8e4 x8T[p, t, m] = X_shard[m, t*128+p] * 0.25
  w8T [128, 2, 4096]   fp8e4 w8T[p, t, n] = W[n, t*128+p] * 4
  bias [4096]          fp32
  out [128, 32, 4096]  bf16  out[p, mi, n] = Y_shard[mi*128+p, n]

Every DMA moves 6-8KB contiguous runs per partition (the DMA engine costs
~19ns/descriptor, so small-element transfers crawl). Inner loop: stationary
= X m-tile, moving = W [128k, 512n]; each half of the n-range accumulates
over k into 4 PSUM banks while the other half's PSUM is evicted (DVE
bias-add + bf16 cast) - ping-pong keeps the PE streaming back-to-back.
"""

import numpy as np
import ml_dtypes

import concourse.bass as bass
import concourse.mybir as mybir
import concourse.tile as tile
from concourse import bacc
from concourse.bass_utils import run_bass_kernel_spmd

P = 128
SEQ, BATCH, D_IN, D_OUT = 8192, 4, 1024, 4096
N_CORES = 8
TOK = SEQ * BATCH
TOK_SHARD = TOK // N_CORES     # 4096
KO = D_IN // P                 # 8 k-tiles total
KF8 = 2                        # k-tiles 0-1 in fp8 DoubleRow
KOB = KO - KF8                 # 6 bf16 k-tiles (real ko 2..7)
M_TILES = TOK_SHARD // P       # 32
NCHUNK = 512                   # moving-operand width (walrus ISA cap)
N_CHUNKS = D_OUT // NCHUNK     # 8
XG = 4                         # m-tiles per X DMA group
G = M_TILES // XG              # 8
X8_SCALE = 0.25                # x*0.25, w*4 -> product unscaled

_CACHE = {}

# Last BassKernelResults, for test harnesses that want exec_time_ns.
LAST_RESULT = None


def _build():
    if "nc" in _CACHE:
        return _CACHE["nc"], _CACHE["names"]

    nc = bacc.Bacc(None, target_bir_lowering=False, debug=False)
    with tile.TileContext(nc) as tc:
        with (
            tc.tile_pool(name="dram", bufs=1, space="DRAM") as dram,
            tc.tile_pool(name="consts", bufs=1) as consts,
            tc.tile_pool(name="opool", bufs=2) as opool,
            tc.tile_pool(name="pspool", bufs=8, space="PSUM") as pspool,
        ):
            xT = dram.tile((G, P, KOB, XG * P), mybir.dt.bfloat16, kind="ExternalInput")
            wT = dram.tile(
                (N_CHUNKS, P, KOB, NCHUNK), mybir.dt.bfloat16, kind="ExternalInput"
            )
            x8T = dram.tile((P, KF8, TOK_SHARD), mybir.dt.float8e4, kind="ExternalInput")
            w8T = dram.tile((P, KF8, D_OUT), mybir.dt.float8e4, kind="ExternalInput")
            bias_d = dram.tile((D_OUT,), mybir.dt.float32, kind="ExternalInput")
            out = dram.tile(
                (P, M_TILES, D_OUT), mybir.dt.bfloat16, kind="ExternalOutput"
            )

            bias_sb = consts.tile([P, D_OUT], mybir.dt.float32, name="bias_sb")
            bias_bcast = bass.AP(
                tensor=bias_d.tensor,
                offset=bias_d.offset,
                ap=[[0, P], *bias_d.ap],
            )

            wc = [None] * N_CHUNKS
            xt = [None] * G

            def load_w(ncix, eng):
                t = consts.tile([P, KOB, NCHUNK], mybir.dt.bfloat16, name=f"w_{ncix}")
                eng.dma_start(out=t[:], in_=wT[ncix])
                wc[ncix] = t

            def load_x(g, eng):
                t = consts.tile([P, KOB, XG * P], mybir.dt.bfloat16, name=f"x_{g}")
                eng.dma_start(out=t[:], in_=xT[g])
                xt[g] = t

            x8 = consts.tile([P, KF8, TOK_SHARD], mybir.dt.float8e4, name="x8")
            w8 = consts.tile([P, KF8, D_OUT], mybir.dt.float8e4, name="w8")

            load_x(0, nc.scalar)
            load_w(0, nc.sync)
            nc.gpsimd.dma_start(out=x8[:], in_=x8T[:])
            nc.gpsimd.dma_start(out=w8[:], in_=w8T[:])
            nc.gpsimd.dma_start(out=bias_sb[:], in_=bias_bcast)
            for ncix in (1, 3, 5, 7):
                load_w(ncix, nc.scalar)
            for ncix in (2, 4, 6):
                load_w(ncix, nc.sync)
            for g in range(1, G):
                load_x(g, nc.sync if g % 2 else nc.scalar)

            H = N_CHUNKS // 2
            for mi in range(M_TILES):
                g, r = divmod(mi, XG)
                ost = opool.tile([P, D_OUT], mybir.dt.bfloat16, name="ost")
                for half in range(2):
                    pss = [
                        pspool.tile([P, NCHUNK], mybir.dt.float32, name="ps")
                        for _ in range(H)
                    ]
                    # fp8 DoubleRow: one K=256 matmul opens each group
                    x8_st = x8[:, :, mi * P : (mi + 1) * P]
                    for j in range(H):
                        ncix = half * H + j
                        nc.tensor.matmul(
                            pss[j][:],
                            x8_st,
                            w8[:, :, ncix * NCHUNK : (ncix + 1) * NCHUNK],
                            start=True,
                            stop=False,
                            perf_mode=mybir.MatmulPerfMode.DoubleRow,
                        )
                    for ko in range(KOB):
                        x_st = xt[g][:, ko, r * P : (r + 1) * P]
                        for j in range(H):
                            ncix = half * H + j
                            nc.tensor.matmul(
                                pss[j][:],
                                x_st,
                                wc[ncix][:, ko, :],
                                start=False,
                                stop=(ko == KOB - 1),
                            )
                    for j in range(H):
                        ncix = half * H + j
                        nc.vector.tensor_add(
                            ost[:, ncix * NCHUNK : (ncix + 1) * NCHUNK],
                            pss[j][:],
                            bias_sb[:, ncix * NCHUNK : (ncix + 1) * NCHUNK],
                        )
                out_eng = nc.sync if mi % 2 else nc.scalar
                out_eng.dma_start(out=out[:, mi, :], in_=ost[:])
    nc.finalize()

    names = (xT.name, wT.name, x8T.name, w8T.name, bias_d.name, out.name)
    _CACHE["nc"] = nc
    _CACHE["names"] = names
    return nc, names


def kernel(x: np.ndarray, weight: np.ndarray, bias: np.ndarray) -> np.ndarray:
    global LAST_RESULT
    nc, (xT_name, wT_name, x8_name, w8_name, bias_name, out_name) = _build()

    x = np.ascontiguousarray(x, dtype=np.float32)
    weight = np.ascontiguousarray(weight, dtype=np.float32)
    bias = np.ascontiguousarray(bias, dtype=np.float32)

    xr = x.reshape(N_CORES, G, XG * P, KO, P)
    # bf16 part: real ko 2..7
    xT_all = np.ascontiguousarray(
        xr[:, :, :, KF8:, :].transpose(0, 1, 4, 3, 2).astype(ml_dtypes.bfloat16)
    )
    # fp8 part: ko 0..1, scaled by 1/4; [c, p, t, m]
    x8_all = np.ascontiguousarray(
        (x.reshape(N_CORES, TOK_SHARD, KO, P)[:, :, :KF8, :] * X8_SCALE)
        .transpose(0, 3, 2, 1)
        .astype(ml_dtypes.float8_e4m3)
    )

    wr = weight.reshape(N_CHUNKS, NCHUNK, KO, P)
    wT_dev = np.ascontiguousarray(
        wr[:, :, KF8:, :].transpose(0, 3, 2, 1).astype(ml_dtypes.bfloat16)
    )
    w8_dev = np.ascontiguousarray(
        (weight.reshape(D_OUT, KO, P)[:, :KF8, :] / X8_SCALE)
        .transpose(2, 1, 0)
        .astype(ml_dtypes.float8_e4m3)
    )

    in_maps = [
        {
            xT_name: xT_all[c],
            wT_name: wT_dev,
            x8_name: x8_all[c],
            w8_name: w8_dev,
            bias_name: bias,
        }
        for c in range(N_CORES)
    ]
    res = run_bass_kernel_spmd(nc, in_maps, list(range(N_CORES)))
    LAST_RESULT = res

    # out[p, mi, n] -> Y_shard[mi*128+p, n]; stack shards along tokens
    y = np.empty((TOK, D_OUT), dtype=np.float32)
    for c in range(N_CORES):
        o = res.results[c][out_name]  # [128, 32, 4096] bf16
        y[c * TOK_SHARD : (c + 1) * TOK_SHARD] = (
            o.astype(np.float32).transpose(1, 0, 2).reshape(TOK_SHARD, D_OUT)
        )
    return y.reshape(SEQ, BATCH, D_OUT)


# revision 17
# speedup vs baseline: 1.1977x; 1.1977x over previous
"""Column-parallel linear Y = X @ W^T + b on 8 Trainium2 NeuronCores.

Strategy: sequence-shard X across the 8 cores (4096 tokens each); every core
holds the full weight, computes its token slab against all 4096 output
features, so no collective is needed and no core re-reads another's tokens.

v4 (mixed fp8/bf16): k-tiles 0-1 (256 of 1024 contraction rows) run as ONE
fp8-e4m3 DoubleRow matmul per output tile (the PE packs 2 fp8 weights per
cell, virtualizing the array to 256x128, streaming 2 k-rows/cycle), k-tiles
2-7 run in bf16. Scales (x*0.25, w*4) cancel in the product so both parts
accumulate into the same PSUM group. Measured norm rel err on the reference
case: 1.61e-2 (gate 2e-2). The output returns bf16; the host upcasts.

Device layout (per core):
  xT  [8, 128, 6, 512] bf16  xT[g, p, ko, m'] = X_shard[g*512+m', (ko+2)*128+p]
  wT  [8, 128, 6, 512] bf16  wT[nc, p, ko, q] = W[nc*512+q, (ko+2)*128+p]
  x8T [128, 2, 4096]   fp8e4 x8T[p, t, m] = X_shard[m, t*128+p] * 0.25
  w8T [128, 2, 4096]   fp8e4 w8T[p, t, n] = W[n, t*128+p] * 4
  bias [4096]          fp32
  out [128, 32, 4096]  bf16  out[p, mi, n] = Y_shard[mi*128+p, n]

Every DMA moves 6-8KB contiguous runs per partition (the DMA engine costs
~19ns/descriptor, so small-element transfers crawl). Inner loop: stationary
= X m-tile, moving = W [128k, 512n]; each half of the n-range accumulates
over k into 4 PSUM banks while the other half's PSUM is evicted (DVE
bias-add + bf16 cast) - ping-pong keeps the PE streaming back-to-back.
"""

import numpy as np
import ml_dtypes

import concourse.bass as bass
import concourse.mybir as mybir
import concourse.tile as tile
from concourse import bacc
from concourse.bass_utils import run_bass_kernel_spmd

P = 128
SEQ, BATCH, D_IN, D_OUT = 8192, 4, 1024, 4096
N_CORES = 8
TOK = SEQ * BATCH
TOK_SHARD = TOK // N_CORES     # 4096
KO = D_IN // P                 # 8 k-tiles total
KF8 = 2                        # k-tiles 0-1 in fp8 DoubleRow
KOB = KO - KF8                 # 6 bf16 k-tiles (real ko 2..7)
M_TILES = TOK_SHARD // P       # 32
NCHUNK = 512                   # moving-operand width (walrus ISA cap)
N_CHUNKS = D_OUT // NCHUNK     # 8
XG = 4                         # m-tiles per X DMA group
G = M_TILES // XG              # 8
X8_SCALE = 0.25                # x*0.25, w*4 -> product unscaled

_CACHE = {}

# Last BassKernelResults, for test harnesses that want exec_time_ns.
LAST_RESULT = None


def _build():
    if "nc" in _CACHE:
        return _CACHE["nc"], _CACHE["names"]

    nc = bacc.Bacc(None, target_bir_lowering=False, debug=False)
    with tile.TileContext(nc) as tc:
        with (
            tc.tile_pool(name="dram", bufs=1, space="DRAM") as dram,
            tc.tile_pool(name="consts", bufs=1) as consts,
            tc.tile_pool(name="opool", bufs=2) as opool,
            tc.tile_pool(name="pspool", bufs=8, space="PSUM") as pspool,
        ):
            xT = dram.tile((G, P, KOB, XG * P), mybir.dt.bfloat16, kind="ExternalInput")
            wT = dram.tile(
                (N_CHUNKS, P, KOB, NCHUNK), mybir.dt.bfloat16, kind="ExternalInput"
            )
            x8T = dram.tile((P, KF8, TOK_SHARD), mybir.dt.float8e4, kind="ExternalInput")
            w8T = dram.tile((P, KF8, D_OUT), mybir.dt.float8e4, kind="ExternalInput")
            bias_d = dram.tile((D_OUT,), mybir.dt.float32, kind="ExternalInput")
            out = dram.tile(
                (P, M_TILES, D_OUT), mybir.dt.bfloat16, kind="ExternalOutput"
            )

            bias_sb = consts.tile([P, D_OUT], mybir.dt.float32, name="bias_sb")
            bias_bcast = bass.AP(
                tensor=bias_d.tensor,
                offset=bias_d.offset,
                ap=[[0, P], *bias_d.ap],
            )
            nc.gpsimd.dma_start(out=bias_sb[:], in_=bias_bcast)

            wc = [None] * N_CHUNKS
            xt = [None] * G

            def load_w(ncix, eng):
                t = consts.tile([P, KOB, NCHUNK], mybir.dt.bfloat16, name=f"w_{ncix}")
                eng.dma_start(out=t[:], in_=wT[ncix])
                wc[ncix] = t

            def load_x(g, eng):
                t = consts.tile([P, KOB, XG * P], mybir.dt.bfloat16, name=f"x_{g}")
                eng.dma_start(out=t[:], in_=xT[g])
                xt[g] = t

            x8 = consts.tile([P, KF8, TOK_SHARD], mybir.dt.float8e4, name="x8")
            w8 = consts.tile([P, KF8, D_OUT], mybir.dt.float8e4, name="w8")

            load_x(0, nc.scalar)
            load_w(0, nc.sync)
            nc.scalar.dma_start(out=x8[:], in_=x8T[:])
            nc.sync.dma_start(out=w8[:], in_=w8T[:])
            for ncix in (1, 3, 5, 7):
                load_w(ncix, nc.scalar)
            for ncix in (2, 4, 6):
                load_w(ncix, nc.sync)
            for g in range(1, G):
                load_x(g, nc.sync if g % 2 else nc.scalar)

            H = N_CHUNKS // 2
            for mi in range(M_TILES):
                g, r = divmod(mi, XG)
                ost = opool.tile([P, D_OUT], mybir.dt.bfloat16, name="ost")
                for half in range(2):
                    pss = [
                        pspool.tile([P, NCHUNK], mybir.dt.float32, name="ps")
                        for _ in range(H)
                    ]
                    # fp8 DoubleRow: one K=256 matmul opens each group
                    x8_st = x8[:, :, mi * P : (mi + 1) * P]
                    for j in range(H):
                        ncix = half * H + j
                        nc.tensor.matmul(
                            pss[j][:],
                            x8_st,
                            w8[:, :, ncix * NCHUNK : (ncix + 1) * NCHUNK],
                            start=True,
                            stop=False,
                            perf_mode=mybir.MatmulPerfMode.DoubleRow,
                        )
                    for ko in range(KOB):
                        x_st = xt[g][:, ko, r * P : (r + 1) * P]
                        for j in range(H):
                            ncix = half * H + j
                            nc.tensor.matmul(
                                pss[j][:],
                                x_st,
                                wc[ncix][:, ko, :],
                                start=False,
                                stop=(ko == KOB - 1),
                            )
                    for j in range(H):
                        ncix = half * H + j
                        nc.vector.tensor_add(
                            ost[:, ncix * NCHUNK : (ncix + 1) * NCHUNK],
                            pss[j][:],
                            bias_sb[:, ncix * NCHUNK : (ncix + 1) * NCHUNK],
                        )
                out_eng = nc.sync if mi % 2 else nc.scalar
                out_eng.dma_start(out=out[:, mi, :], in_=ost[:])
    nc.finalize()

    names = (xT.name, wT.name, x8T.name, w8T.name, bias_d.name, out.name)
    _CACHE["nc"] = nc
    _CACHE["names"] = names
    return nc, names


def kernel(x: np.ndarray, weight: np.ndarray, bias: np.ndarray) -> np.ndarray:
    global LAST_RESULT
    nc, (xT_name, wT_name, x8_name, w8_name, bias_name, out_name) = _build()

    x = np.ascontiguousarray(x, dtype=np.float32)
    weight = np.ascontiguousarray(weight, dtype=np.float32)
    bias = np.ascontiguousarray(bias, dtype=np.float32)

    xr = x.reshape(N_CORES, G, XG * P, KO, P)
    # bf16 part: real ko 2..7
    xT_all = np.ascontiguousarray(
        xr[:, :, :, KF8:, :].transpose(0, 1, 4, 3, 2).astype(ml_dtypes.bfloat16)
    )
    # fp8 part: ko 0..1, scaled by 1/4; [c, p, t, m]
    x8_all = np.ascontiguousarray(
        (x.reshape(N_CORES, TOK_SHARD, KO, P)[:, :, :KF8, :] * X8_SCALE)
        .transpose(0, 3, 2, 1)
        .astype(ml_dtypes.float8_e4m3)
    )

    wr = weight.reshape(N_CHUNKS, NCHUNK, KO, P)
    wT_dev = np.ascontiguousarray(
        wr[:, :, KF8:, :].transpose(0, 3, 2, 1).astype(ml_dtypes.bfloat16)
    )
    w8_dev = np.ascontiguousarray(
        (weight.reshape(D_OUT, KO, P)[:, :KF8, :] / X8_SCALE)
        .transpose(2, 1, 0)
        .astype(ml_dtypes.float8_e4m3)
    )

    in_maps = [
        {
            xT_name: xT_all[c],
            wT_name: wT_dev,
            x8_name: x8_all[c],
            w8_name: w8_dev,
            bias_name: bias,
        }
        for c in range(N_CORES)
    ]
    res = run_bass_kernel_spmd(nc, in_maps, list(range(N_CORES)))
    LAST_RESULT = res

    # out[p, mi, n] -> Y_shard[mi*128+p, n]; stack shards along tokens
    y = np.empty((TOK, D_OUT), dtype=np.float32)
    for c in range(N_CORES):
        o = res.results[c][out_name]  # [128, 32, 4096] bf16
        y[c * TOK_SHARD : (c + 1) * TOK_SHARD] = (
            o.astype(np.float32).transpose(1, 0, 2).reshape(TOK_SHARD, D_OUT)
        )
    return y.reshape(SEQ, BATCH, D_OUT)
